# revision 8
# baseline (speedup 1.0000x reference)
"""Bass/Tile Trainium2 kernel for CrossPositionalAttention.

Reference math (per batch element b):
    M = F @ W_M; N = F @ W_N; V = F @ W_V          # [T, C] each, T=2048, C=64
    S = softmax(M @ N^T, axis=-1)                  # [T, T]
    out = S @ V + F

Sharding: data-parallel over batch. B=8 == n_cores=8, so core i computes
batch element i end-to-end (no collectives); kernel() shards/gathers on host.

Key structure (per core, P=128 partitions):
  Weight fusion: scores = M N^T = F (W_M W_N^T) F^T, so the host passes
    G = W_M W_N^T and the kernel computes P^T = G^T F^T once ([64, T]);
    scores^T tiles are then F_T_blk^T @ P^T -- no N projection at all.
  fp16 operands: F^T and P^T are stored fp16 (1 cyc/col PE streaming like
    bf16, but 10 mantissa bits; values are O(10) so range is safe).  expS
    and V are bf16 (exp(s-40) reaches e^29, beyond fp16 range).
  Mainloop (per q-chunk of 512, per kp pair of k-blocks): two k=64 scores
    matmuls [64,128]x[64,512] -> PSUM [128, 1024]; ACT exp (bias -40,
    shift-invariant softmax, scores stay in fp32 range) -> bf16 expS; PV
    matmuls (lagged one iteration) accumulate [66, 512] f32: V rows plus
    a ones-column that yields the softmax denominator.
  ACT is the pacing engine (~1147ns per [128,1024] exp).  The PE has
    slack each iteration; dummy "filler" matmuls keep its activity window
    saturated, because sustained micro-idle lets the HAM clock gate
    demote the PE to 1.2 GHz -- observed to then stick at half clock for
    the rest of the kernel (re-promotion never fires mid-kernel).
  Head: an UNINTERRUPTED ~3.5us warm-up matmul burst (dummies only -- a
    DMA-wait inside the burst resets the HAM activity window) trips the
    clock gate to K=8/8 while the F DMAs land; phase-A groups 0/1 run
    right after it, groups 2/3 are spread over qc=0 iterations.
  Per-chunk epilogue (transpose pv -> [128,66], out = pv[:, :64] *
    recip(pv[:, 64]) + F) is interleaved one block per iteration into the
    NEXT q-chunk so the PE never bursts at chunk boundaries; the last
    chunk pipelines per-block across engines and both DMA queues.
"""

import numpy as np

import concourse.bacc as bacc
import concourse.bass as bass
import concourse.tile as tile
from concourse import mybir
from concourse.bass_utils import run_bass_kernel_spmd
from concourse.masks import make_identity

B, T, C = 8, 2048, 64
P = 128
NBLK = T // P          # 16 k-blocks (and q-blocks) of 128
QCHUNK = 512           # moving-operand free dim per matmul
NQC = T // QCHUNK      # 4 q-chunks
F32 = mybir.dt.float32
BF16 = mybir.dt.bfloat16
FP16 = mybir.dt.float16
F32R = mybir.dt.float32r
EXP_BIAS = -40.0       # constant softmax shift (cancels in the normalization)
VPAD = 66              # V tile free dim: 64 V cols + ones col + pad (even)
NWARM = 8              # uninterrupted head warm-up burst (8 x ~427ns cold)


def build_nc() -> bass.Bass:
    nc = bacc.Bacc()
    F_h = nc.declare_dram_parameter("F", [T, C], F32, isOutput=False)
    G_h = nc.declare_dram_parameter("G", [C, C], F32, isOutput=False)
    Wv_h = nc.declare_dram_parameter("W_V", [C, C], F32, isOutput=False)
    out_h = nc.declare_dram_parameter("out", [T, C], F32, isOutput=True)

    # [T, C] viewed as [128, 16, C]: partition p, block n -> row n*128 + p
    F_view = F_h[:, :].rearrange("(n p) c -> p n c", p=P)
    out_view = out_h[:, :].rearrange("(n p) c -> p n c", p=P)

    with tile.TileContext(nc) as tc:
        with (
            tc.tile_pool(name="const", bufs=1) as const_pool,
            tc.tile_pool(name="persist", bufs=1) as persist,
            tc.tile_pool(name="mix_ps", bufs=2, space="PSUM") as mix_ps,
            tc.tile_pool(name="sc_ps", bufs=2, space="PSUM") as sc_pool,
            tc.tile_pool(name="pv_ps", bufs=2, space="PSUM") as pv_pool,
            tc.tile_pool(name="work", bufs=4) as work,
            tc.tile_pool(name="ep", bufs=4) as ep,
            tc.tile_pool(name="opool", bufs=2) as opool,
        ):
            # warm tile on DVE (fast) so the burst starts ASAP
            warm = const_pool.tile([P, P + QCHUNK], BF16, tag="warm")
            nc.vector.memset(warm, 0.25)

            # each dma_start costs ~650ns of fixed sequencer issue time, so
            # F goes out as ONE descriptor batch on sync; weights on scalar
            F_sb = persist.tile([P, NBLK, C], F32, tag="fsb")
            nc.sync.dma_start(out=F_sb, in_=F_view)

            Wstage = const_pool.tile([C, 2, C], F32, tag="wstage")
            nc.scalar.dma_start(out=Wstage[:, 0, :], in_=G_h[:, :])
            nc.scalar.dma_start(out=Wstage[:, 1, :], in_=Wv_h[:, :])

            G_sb = const_pool.tile([C, C], FP16, tag="gsb")
            Wv_sb = const_pool.tile([C, C], FP16, tag="wv")
            nc.vector.tensor_copy(G_sb, Wstage[:, 0, :])
            nc.vector.tensor_copy(Wv_sb, Wstage[:, 1, :])

            ident = const_pool.tile([P, P], F32, tag="ident")
            make_identity(nc, ident)
            ident_r = const_pool.tile([P, P], F32R, tag="identr")
            nc.vector.tensor_copy(ident_r, ident)

            exp_bias = const_pool.tile([P, 1], F32, tag="expbias")
            nc.vector.memset(exp_bias, EXP_BIAS)
            # preload the exp ACT table while DMAs land (issued on the scalar
            # queue after its F dma_starts; ~2.7us table load off critical path)
            tbl_dummy = const_pool.tile([P, 1], F32, tag="tbldummy")
            nc.scalar.activation(
                tbl_dummy, exp_bias, mybir.ActivationFunctionType.Exp
            )

            F_T = persist.tile([C, T], FP16, tag="ft")
            PT = persist.tile([C, T], FP16, tag="pt")
            # PV path in bf16 (fp16 cannot hold exp(s-40) up to e^29)
            V_sb = persist.tile([P, NBLK, VPAD], BF16, tag="vsb")
            # ones col 64 of every V block -> softmax denominator via PV
            nc.vector.memset(V_sb[:, :, C:VPAD], 1.0)

            def warm_mm(n=QCHUNK):
                # dummy bf16 matmul: counts as real PE activity for the
                # HAM clock gate (transposes don't)
                wps = mix_ps.tile([P, QCHUNK], F32, tag="mix", name="wps")
                nc.tensor.matmul(
                    wps,
                    lhsT=warm[:, 0:P],
                    rhs=warm[:, P : P + n],
                    start=True,
                    stop=True,
                )

            def prep_t(g):
                """F^T transposes for k-blocks 4g..4g+3 (fp16 out)."""
                for pair in range(2):
                    tp = mix_ps.tile([C, 2, P], F32, tag="mix", name="tp")
                    n0 = 4 * g + 2 * pair
                    for i in range(2):
                        nc.tensor.transpose(
                            tp[:, i, :], F_sb[:, n0 + i, :], ident
                        )
                    # PSUM f32 -> SBUF fp16 copy performs the rounding
                    nc.vector.tensor_copy(F_T[:, n0 * P : (n0 + 2) * P], tp)

            def prep_p(g):
                """P^T = G^T F^T chunk g."""
                sl = slice(g * QCHUNK, (g + 1) * QCHUNK)
                pp = mix_ps.tile([C, QCHUNK], F32, tag="mix", name="pp")
                nc.tensor.matmul(
                    pp, lhsT=G_sb, rhs=F_T[:, sl], start=True, stop=True
                )
                nc.vector.tensor_copy(PT[:, sl], pp)

            def prep_v(g):
                """V = F W_V blocks 4g..4g+3 (bf16 out)."""
                vp = mix_ps.tile([P, 4, C], F32, tag="mix", name="vp")
                for i in range(4):
                    n = 4 * g + i
                    nc.tensor.matmul(
                        vp[:, i, :],
                        lhsT=F_T[:, n * P : (n + 1) * P],
                        rhs=Wv_sb,
                        start=True,
                        stop=True,
                    )
                nc.vector.tensor_copy(V_sb[:, 4 * g : 4 * g + 4, 0:C], vp)

            # uninterrupted warm-up burst (~3.5us cold) trips the HAM gate;
            # by its end the F DMAs have landed, so groups 0/1 run warm
            for _ in range(NWARM):
                warm_mm()
            for g in (0, 1):
                prep_t(g)
                prep_p(g)
                prep_v(g)

            def ep_block(pv_sb_p, o_sb_p, qcp, j):
                """One 128-q block of the chunk-qcp epilogue (steady chunks)."""
                qb = qcp * (QCHUNK // P) + j
                trr = mix_ps.tile([P, VPAD], F32R, tag="mix", name="trr")
                nc.tensor.transpose(
                    trr,
                    pv_sb_p[:, j * P : (j + 1) * P],
                    ident_r[0:VPAD, 0:VPAD],
                )
                tr = trr.bitcast(F32)
                rcp = ep.tile([P, 1], F32, tag="rcp")
                nc.vector.reciprocal(rcp, tr[:, C : C + 1])
                nc.vector.tensor_scalar_mul(o_sb_p[:, j, :], tr[:, 0:C], rcp)
                nc.vector.tensor_add(
                    o_sb_p[:, j, :], o_sb_p[:, j, :], F_sb[:, qb, :]
                )
                if j == 3:
                    nc.sync.dma_start(
                        out=out_view[:, qcp * NQC : (qcp + 1) * NQC, :],
                        in_=o_sb_p,
                    )

            prev_ep = None  # (pv_sb, o_sb, qc) of the chunk awaiting epilogue
            for qc in range(NQC):
                qsl = slice(qc * QCHUNK, (qc + 1) * QCHUNK)
                pv_ps = pv_pool.tile([VPAD, QCHUNK], F32, tag="pv")
                pend = None  # software-pipelined PV (lags scores by 1)
                for kp in range(NBLK // 2):
                    # PE filler sizing: ACT paces at ~1147ns/iter; fill PE
                    # slack so the HAM activity window stays saturated.
                    # qc=0 gets no fillers -- its slack is filled by the
                    # phase-A groups, and unanchored fillers float in the
                    # schedule and delay real work on the FIFO PE queue.
                    n_fill = 0
                    if qc > 0 and kp >= 4:
                        n_fill = 1
                    for _ in range(n_fill):
                        warm_mm()
                    sc_ps = sc_pool.tile([P, 2 * QCHUNK], F32, tag="sc")
                    # scores^T for k-blocks 2kp / 2kp+1 (k=64 contraction)
                    for h, kblk in ((0, 2 * kp), (1, 2 * kp + 1)):
                        ksl = slice(kblk * P, (kblk + 1) * P)
                        bank = slice(h * QCHUNK, (h + 1) * QCHUNK)
                        nc.tensor.matmul(
                            sc_ps[:, bank],
                            lhsT=F_T[:, ksl],
                            rhs=PT[:, qsl],
                            start=True,
                            stop=True,
                        )
                    if pend is not None:
                        for h in range(2):
                            nc.tensor.matmul(
                                pv_ps,
                                lhsT=V_sb[:, 2 * pend[0] + h, :],
                                rhs=pend[1][:, h * QCHUNK : (h + 1) * QCHUNK],
                                start=(pend[0] == 0 and h == 0),
                                stop=False,
                            )
                    # previous chunk's epilogue: one block per iteration
                    if prev_ep is not None and kp < 4:
                        ep_block(prev_ep[0], prev_ep[1], prev_ep[2], kp)
                    expS = work.tile([P, 2 * QCHUNK], BF16, tag="exps")
                    if qc == NQC - 1 and kp == NBLK // 2 - 1:
                        # tail: split the last exp so the final PV pair
                        # starts after the first half (subtile deps)
                        for h in range(2):
                            hsl = slice(h * QCHUNK, (h + 1) * QCHUNK)
                            nc.scalar.activation(
                                expS[:, hsl],
                                sc_ps[:, hsl],
                                mybir.ActivationFunctionType.Exp,
                                bias=exp_bias,
                                scale=1.0,
                            )
                    else:
                        nc.scalar.activation(
                            expS,
                            sc_ps,
                            mybir.ActivationFunctionType.Exp,
                            bias=exp_bias,
                            scale=1.0,
                        )
                    pend = (kp, expS)
                    # phase-A groups 2/3 spread across early qc=0
                    # iterations (emitted after scores/PV so they fill the
                    # PE slack instead of delaying the exp pipeline)
                    if qc == 0:
                        if kp == 0:
                            prep_t(2)
                        elif kp == 1:
                            prep_p(2)
                            prep_v(2)
                        elif kp == 2:
                            prep_t(3)
                        elif kp == 3:
                            prep_p(3)
                            prep_v(3)
                for h in range(2):
                    nc.tensor.matmul(
                        pv_ps,
                        lhsT=V_sb[:, 2 * pend[0] + h, :],
                        rhs=pend[1][:, h * QCHUNK : (h + 1) * QCHUNK],
                        start=False,
                        stop=(h == 1),
                    )

                pv_sb = ep.tile([VPAD, QCHUNK], F32R, tag="pvsb")
                last = qc == NQC - 1
                if last:
                    # tail: quarter the copy so the first transpose
                    # starts after 1/4 of the data is in SBUF
                    for q4 in range(4):
                        sl4 = slice(q4 * P, (q4 + 1) * P)
                        nc.vector.tensor_copy(pv_sb[:, sl4], pv_ps[:, sl4])
                else:
                    nc.vector.tensor_copy(pv_sb, pv_ps)
                o_sb = opool.tile([P, NQC, C], F32, tag="osb")
                if not last:
                    prev_ep = (pv_sb, o_sb, qc)
                    continue
                # ---- tail: last chunk pipelines per-block across engines ----
                for j in range(QCHUNK // P):
                    qb = qc * (QCHUNK // P) + j
                    trr = mix_ps.tile([P, VPAD], F32R, tag="mix", name="trr")
                    nc.tensor.transpose(
                        trr,
                        pv_sb[:, j * P : (j + 1) * P],
                        ident_r[0:VPAD, 0:VPAD],
                    )
                    tr = trr.bitcast(F32)
                    rcp = ep.tile([P, 1], F32, tag="rcp")
                    nc.vector.reciprocal(rcp, tr[:, C : C + 1])
                    # spread the chain across three engines -- muls on
                    # Scalar (idle after the last exp) + DVE, residual
                    # adds on GpSimd (SBUF-only, allowed)
                    if j % 2 == 0:
                        nc.scalar.activation(
                            o_sb[:, j, :],
                            tr[:, 0:C],
                            mybir.ActivationFunctionType.Copy,
                            scale=rcp,
                        )
                    else:
                        nc.vector.tensor_scalar_mul(
                            o_sb[:, j, :], tr[:, 0:C], rcp
                        )
                    nc.gpsimd.tensor_tensor(
                        out=o_sb[:, j, :],
                        in0=o_sb[:, j, :],
                        in1=F_sb[:, qb, :],
                        op=mybir.AluOpType.add,
                    )
                    # per-block DMAs; the last one goes on the scalar
                    # queue so issues overlap
                    eng = nc.scalar if j == 3 else nc.sync
                    eng.dma_start(out=out_view[:, qb, :], in_=o_sb[:, j, :])

    nc.finalize()
    return nc


_NC_CACHE = None


def _get_nc() -> bass.Bass:
    global _NC_CACHE
    if _NC_CACHE is None:
        _NC_CACHE = build_nc()
    return _NC_CACHE


def run_spmd(F, W_M, W_N, W_V, **kwargs):
    """Run the SPMD kernel; returns the BassKernelResults (for profiling)."""
    nc = _get_nc()
    G = np.ascontiguousarray(
        W_M.astype(np.float32) @ W_N.astype(np.float32).T
    )
    in_maps = [
        {
            "F": np.ascontiguousarray(F[i], dtype=np.float32),
            "G": G,
            "W_V": np.ascontiguousarray(W_V, dtype=np.float32),
        }
        for i in range(B)
    ]
    return run_bass_kernel_spmd(nc, in_maps, core_ids=list(range(B)), **kwargs)


def kernel(F, W_M, W_N, W_V):
    res = run_spmd(F, W_M, W_N, W_V)
    return np.stack([r["out"] for r in res.results]).astype(np.float32)


# revision 10
# speedup vs baseline: 1.0196x; 1.0196x over previous
"""Bass/Tile Trainium2 kernel for CrossPositionalAttention.

Reference math (per batch element b):
    M = F @ W_M; N = F @ W_N; V = F @ W_V          # [T, C] each, T=2048, C=64
    S = softmax(M @ N^T, axis=-1)                  # [T, T]
    out = S @ V + F

Sharding: data-parallel over batch. B=8 == n_cores=8, so core i computes
batch element i end-to-end (no collectives); kernel() shards/gathers on host.

Key structure (per core, P=128 partitions):
  Weight fusion: scores = M N^T = F (W_M W_N^T) F^T, so the host passes
    G = W_M W_N^T and the kernel computes P^T = G^T F^T once ([64, T]);
    scores^T tiles are then F_T_blk^T @ P^T -- no N projection at all.
  fp16 operands: F^T and P^T are stored fp16 (1 cyc/col PE streaming like
    bf16, but 10 mantissa bits; values are O(10) so range is safe).  expS
    and V are bf16 (exp(s-40) reaches e^29, beyond fp16 range).
  Mainloop (per q-chunk of 512, per kp pair of k-blocks): two k=64 scores
    matmuls [64,128]x[64,512] -> PSUM [128, 1024]; ACT exp (bias -40,
    shift-invariant softmax, scores stay in fp32 range) -> bf16 expS; PV
    matmuls (lagged one iteration) accumulate [66, 512] f32: V rows plus
    a ones-column that yields the softmax denominator.
  ACT is the pacing engine (~1147ns per [128,1024] exp).  The PE has
    slack each iteration; dummy "filler" matmuls keep its activity window
    saturated, because sustained micro-idle lets the HAM clock gate
    demote the PE to 1.2 GHz -- observed to then stick at half clock for
    the rest of the kernel (re-promotion never fires mid-kernel).
  Head: an UNINTERRUPTED ~3.5us warm-up matmul burst (dummies only -- a
    DMA-wait inside the burst resets the HAM activity window) trips the
    clock gate to K=8/8 while the F DMAs land; phase-A groups 0/1 run
    right after it, groups 2/3 are spread over qc=0 iterations.
  Per-chunk epilogue (transpose pv -> [128,66], out = pv[:, :64] *
    recip(pv[:, 64]) + F) is interleaved one block per iteration into the
    NEXT q-chunk so the PE never bursts at chunk boundaries; the last
    chunk pipelines per-block across engines and both DMA queues.
"""

import numpy as np

import concourse.bacc as bacc
import concourse.bass as bass
import concourse.tile as tile
from concourse import mybir
from concourse.bass_utils import run_bass_kernel_spmd
from concourse.masks import make_identity

B, T, C = 8, 2048, 64
P = 128
NBLK = T // P          # 16 k-blocks (and q-blocks) of 128
QCHUNK = 512           # moving-operand free dim per matmul
NQC = T // QCHUNK      # 4 q-chunks
F32 = mybir.dt.float32
BF16 = mybir.dt.bfloat16
FP16 = mybir.dt.float16
F32R = mybir.dt.float32r
EXP_BIAS = -40.0       # constant softmax shift (cancels in the normalization)
VPAD = 66              # V tile free dim: 64 V cols + ones col + pad (even)
NWARM = 8              # uninterrupted head warm-up burst (8 x ~427ns cold)


def build_nc() -> bass.Bass:
    nc = bacc.Bacc()
    F_h = nc.declare_dram_parameter("F", [T, C], F32, isOutput=False)
    G_h = nc.declare_dram_parameter("G", [C, C], F32, isOutput=False)
    Wv_h = nc.declare_dram_parameter("W_V", [C, C], F32, isOutput=False)
    out_h = nc.declare_dram_parameter("out", [T, C], F32, isOutput=True)

    # [T, C] viewed as [128, 16, C]: partition p, block n -> row n*128 + p
    F_view = F_h[:, :].rearrange("(n p) c -> p n c", p=P)
    out_view = out_h[:, :].rearrange("(n p) c -> p n c", p=P)

    with tile.TileContext(nc) as tc:
        with (
            tc.tile_pool(name="const", bufs=1) as const_pool,
            tc.tile_pool(name="persist", bufs=1) as persist,
            tc.tile_pool(name="mix_ps", bufs=2, space="PSUM") as mix_ps,
            tc.tile_pool(name="sc_ps", bufs=2, space="PSUM") as sc_pool,
            tc.tile_pool(name="pv_ps", bufs=2, space="PSUM") as pv_pool,
            tc.tile_pool(name="work", bufs=4) as work,
            tc.tile_pool(name="ep", bufs=4) as ep,
            tc.tile_pool(name="opool", bufs=2) as opool,
        ):
            # warm tile on DVE (fast) so the burst starts ASAP
            warm = const_pool.tile([P, P + QCHUNK], BF16, tag="warm")
            nc.vector.memset(warm, 0.25)

            # each dma_start costs ~650ns of fixed sequencer issue time, so
            # F goes out as ONE descriptor batch on sync; weights on scalar
            F_sb = persist.tile([P, NBLK, C], F32, tag="fsb")
            nc.sync.dma_start(out=F_sb, in_=F_view)

            Wstage = const_pool.tile([C, 2, C], F32, tag="wstage")
            nc.scalar.dma_start(out=Wstage[:, 0, :], in_=G_h[:, :])
            nc.scalar.dma_start(out=Wstage[:, 1, :], in_=Wv_h[:, :])

            G_sb = const_pool.tile([C, C], FP16, tag="gsb")
            Wv_sb = const_pool.tile([C, C], FP16, tag="wv")
            nc.vector.tensor_copy(G_sb, Wstage[:, 0, :])
            nc.vector.tensor_copy(Wv_sb, Wstage[:, 1, :])

            ident = const_pool.tile([P, P], F32, tag="ident")
            make_identity(nc, ident)
            ident_r = const_pool.tile([P, P], F32R, tag="identr")
            nc.vector.tensor_copy(ident_r, ident)

            exp_bias = const_pool.tile([P, 1], F32, tag="expbias")
            nc.vector.memset(exp_bias, EXP_BIAS)
            # preload the exp ACT table while DMAs land (issued on the scalar
            # queue after its F dma_starts; ~2.7us table load off critical path)
            tbl_dummy = const_pool.tile([P, 1], F32, tag="tbldummy")
            nc.scalar.activation(
                tbl_dummy, exp_bias, mybir.ActivationFunctionType.Exp
            )

            F_T = persist.tile([C, T], FP16, tag="ft")
            PT = persist.tile([C, T], FP16, tag="pt")
            # PV path in bf16 (fp16 cannot hold exp(s-40) up to e^29)
            V_sb = persist.tile([P, NBLK, VPAD], BF16, tag="vsb")
            # ones col 64 of every V block -> softmax denominator via PV
            nc.vector.memset(V_sb[:, :, C:VPAD], 1.0)

            def warm_mm(n=QCHUNK):
                # dummy bf16 matmul: counts as real PE activity for the
                # HAM clock gate (transposes don't)
                wps = mix_ps.tile([P, QCHUNK], F32, tag="mix", name="wps")
                nc.tensor.matmul(
                    wps,
                    lhsT=warm[:, 0:P],
                    rhs=warm[:, P : P + n],
                    start=True,
                    stop=True,
                )

            def prep_t(g):
                """F^T transposes for k-blocks 4g..4g+3 (fp16 out)."""
                for pair in range(2):
                    tp = mix_ps.tile([C, 2, P], F32, tag="mix", name="tp")
                    n0 = 4 * g + 2 * pair
                    for i in range(2):
                        nc.tensor.transpose(
                            tp[:, i, :], F_sb[:, n0 + i, :], ident
                        )
                    # PSUM f32 -> SBUF fp16 copy performs the rounding
                    nc.vector.tensor_copy(F_T[:, n0 * P : (n0 + 2) * P], tp)

            def prep_p(g):
                """P^T = G^T F^T chunk g."""
                sl = slice(g * QCHUNK, (g + 1) * QCHUNK)
                pp = mix_ps.tile([C, QCHUNK], F32, tag="mix", name="pp")
                nc.tensor.matmul(
                    pp, lhsT=G_sb, rhs=F_T[:, sl], start=True, stop=True
                )
                nc.vector.tensor_copy(PT[:, sl], pp)

            def prep_v(g):
                """V = F W_V blocks 4g..4g+3 (bf16 out)."""
                vp = mix_ps.tile([P, 4, C], F32, tag="mix", name="vp")
                for i in range(4):
                    n = 4 * g + i
                    nc.tensor.matmul(
                        vp[:, i, :],
                        lhsT=F_T[:, n * P : (n + 1) * P],
                        rhs=Wv_sb,
                        start=True,
                        stop=True,
                    )
                nc.vector.tensor_copy(V_sb[:, 4 * g : 4 * g + 4, 0:C], vp)

            # uninterrupted warm-up burst (~3.5us cold) trips the HAM gate;
            # by its end the F DMAs have landed, so groups 0/1 run warm
            for _ in range(NWARM):
                warm_mm()
            for g in (0, 1):
                prep_t(g)
                prep_p(g)
                prep_v(g)

            def ep_block(pv_sb_p, o_sb_p, qcp, j):
                """One 128-q block of the chunk-qcp epilogue (steady chunks)."""
                qb = qcp * (QCHUNK // P) + j
                trr = mix_ps.tile([P, VPAD], F32R, tag="mix", name="trr")
                nc.tensor.transpose(
                    trr,
                    pv_sb_p[:, j * P : (j + 1) * P],
                    ident_r[0:VPAD, 0:VPAD],
                )
                tr = trr.bitcast(F32)
                rcp = ep.tile([P, 1], F32, tag="rcp")
                nc.vector.reciprocal(rcp, tr[:, C : C + 1])
                nc.vector.tensor_scalar_mul(o_sb_p[:, j, :], tr[:, 0:C], rcp)
                nc.vector.tensor_add(
                    o_sb_p[:, j, :], o_sb_p[:, j, :], F_sb[:, qb, :]
                )
                if j == 3:
                    nc.sync.dma_start(
                        out=out_view[:, qcp * NQC : (qcp + 1) * NQC, :],
                        in_=o_sb_p,
                    )

            prev_ep = None  # (pv_sb, o_sb, qc) of the chunk awaiting epilogue
            for qc in range(NQC):
                qsl = slice(qc * QCHUNK, (qc + 1) * QCHUNK)
                pv_ps = pv_pool.tile([VPAD, QCHUNK], F32, tag="pv")
                pend = None  # software-pipelined PV (lags scores by 1)
                for kp in range(NBLK // 2):
                    sc_ps = sc_pool.tile([P, 2 * QCHUNK], F32, tag="sc")
                    # scores^T for k-blocks 2kp / 2kp+1 (k=64 contraction).
                    # Scores go FIRST on the PE queue (they feed the exp
                    # pipeline); the PV pair goes LAST because it blocks on
                    # exp(kp-1) and the PE executes its queue in FIFO order
                    # -- ready work must sit ahead of the blocking PV.
                    for h, kblk in ((0, 2 * kp), (1, 2 * kp + 1)):
                        ksl = slice(kblk * P, (kblk + 1) * P)
                        bank = slice(h * QCHUNK, (h + 1) * QCHUNK)
                        nc.tensor.matmul(
                            sc_ps[:, bank],
                            lhsT=F_T[:, ksl],
                            rhs=PT[:, qsl],
                            start=True,
                            stop=True,
                        )
                    # phase-A groups 2/3 spread across qc=0 iterations
                    if qc == 0:
                        if kp == 1:
                            prep_t(2)
                        elif kp == 2:
                            prep_p(2)
                            prep_v(2)
                        elif kp == 3:
                            prep_t(3)
                        elif kp == 4:
                            prep_p(3)
                            prep_v(3)
                    # previous chunk's epilogue: one block per iteration
                    if prev_ep is not None and kp < 4:
                        ep_block(prev_ep[0], prev_ep[1], prev_ep[2], kp)
                    # PE filler: keeps the HAM activity window saturated
                    n_fill = 1
                    if qc == 0 and kp in (1, 2, 3, 4):
                        n_fill = 0
                    elif qc > 0 and kp < 4:
                        n_fill = 0
                    for _ in range(n_fill):
                        warm_mm()
                    if pend is not None:
                        for h in range(2):
                            nc.tensor.matmul(
                                pv_ps,
                                lhsT=V_sb[:, 2 * pend[0] + h, :],
                                rhs=pend[1][:, h * QCHUNK : (h + 1) * QCHUNK],
                                start=(pend[0] == 0 and h == 0),
                                stop=False,
                            )
                    expS = work.tile([P, 2 * QCHUNK], BF16, tag="exps")
                    if qc == NQC - 1 and kp == NBLK // 2 - 1:
                        # tail: split the last exp so the final PV pair
                        # starts after the first half (subtile deps)
                        for h in range(2):
                            hsl = slice(h * QCHUNK, (h + 1) * QCHUNK)
                            nc.scalar.activation(
                                expS[:, hsl],
                                sc_ps[:, hsl],
                                mybir.ActivationFunctionType.Exp,
                                bias=exp_bias,
                                scale=1.0,
                            )
                    else:
                        nc.scalar.activation(
                            expS,
                            sc_ps,
                            mybir.ActivationFunctionType.Exp,
                            bias=exp_bias,
                            scale=1.0,
                        )
                    pend = (kp, expS)
                for h in range(2):
                    nc.tensor.matmul(
                        pv_ps,
                        lhsT=V_sb[:, 2 * pend[0] + h, :],
                        rhs=pend[1][:, h * QCHUNK : (h + 1) * QCHUNK],
                        start=False,
                        stop=(h == 1),
                    )

                pv_sb = ep.tile([VPAD, QCHUNK], F32R, tag="pvsb")
                last = qc == NQC - 1
                if last:
                    # tail: quarter the copy so the first transpose
                    # starts after 1/4 of the data is in SBUF
                    for q4 in range(4):
                        sl4 = slice(q4 * P, (q4 + 1) * P)
                        nc.vector.tensor_copy(pv_sb[:, sl4], pv_ps[:, sl4])
                else:
                    nc.vector.tensor_copy(pv_sb, pv_ps)
                o_sb = opool.tile([P, NQC, C], F32, tag="osb")
                if not last:
                    prev_ep = (pv_sb, o_sb, qc)
                    continue
                # ---- tail: last chunk pipelines per-block across engines ----
                for j in range(QCHUNK // P):
                    qb = qc * (QCHUNK // P) + j
                    trr = mix_ps.tile([P, VPAD], F32R, tag="mix", name="trr")
                    nc.tensor.transpose(
                        trr,
                        pv_sb[:, j * P : (j + 1) * P],
                        ident_r[0:VPAD, 0:VPAD],
                    )
                    tr = trr.bitcast(F32)
                    rcp = ep.tile([P, 1], F32, tag="rcp")
                    nc.vector.reciprocal(rcp, tr[:, C : C + 1])
                    # spread the chain across three engines -- muls on
                    # Scalar (idle after the last exp) + DVE, residual
                    # adds on GpSimd (SBUF-only, allowed)
                    if j % 2 == 0:
                        nc.scalar.activation(
                            o_sb[:, j, :],
                            tr[:, 0:C],
                            mybir.ActivationFunctionType.Copy,
                            scale=rcp,
                        )
                    else:
                        nc.vector.tensor_scalar_mul(
                            o_sb[:, j, :], tr[:, 0:C], rcp
                        )
                    nc.gpsimd.tensor_tensor(
                        out=o_sb[:, j, :],
                        in0=o_sb[:, j, :],
                        in1=F_sb[:, qb, :],
                        op=mybir.AluOpType.add,
                    )
                    # per-block DMAs; the last one goes on the scalar
                    # queue so issues overlap
                    eng = nc.scalar if j == 3 else nc.sync
                    eng.dma_start(out=out_view[:, qb, :], in_=o_sb[:, j, :])

    nc.finalize()
    return nc


_NC_CACHE = None


def _get_nc() -> bass.Bass:
    global _NC_CACHE
    if _NC_CACHE is None:
        _NC_CACHE = build_nc()
    return _NC_CACHE


def run_spmd(F, W_M, W_N, W_V, **kwargs):
    """Run the SPMD kernel; returns the BassKernelResults (for profiling)."""
    nc = _get_nc()
    G = np.ascontiguousarray(
        W_M.astype(np.float32) @ W_N.astype(np.float32).T
    )
    in_maps = [
        {
            "F": np.ascontiguousarray(F[i], dtype=np.float32),
            "G": G,
            "W_V": np.ascontiguousarray(W_V, dtype=np.float32),
        }
        for i in range(B)
    ]
    return run_bass_kernel_spmd(nc, in_maps, core_ids=list(range(B)), **kwargs)


def kernel(F, W_M, W_N, W_V):
    res = run_spmd(F, W_M, W_N, W_V)
    return np.stack([r["out"] for r in res.results]).astype(np.float32)
